# revision 11
# baseline (speedup 1.0000x reference)
"""v3: PE-accumulated window sums ("layout B", j-banks variant).

Same sharding and math as v2 (see kernel.py), but the window reduction
Sum_t E_t and Sum_t E_t*v_t runs on the TensorEngine instead of DVE adds:

- Partitions = j-slot x 32 channels.  Bank A: j in 0..3 (128 partitions),
  bank B: j in 4..6 (96 partitions).
- k_unf/v_unf hold the j-pre-shifted padded [70x70] grids, so window offset
  (i, j) is a pure row shift (i) of partition (j, c)'s grid.
- Per i and bank: one fused STT logit op, one batched exp (ACT), one E*v
  multiply, then 16 block-identity matmuls (8 chunks x {D, N}) that
  accumulate straight into PSUM across all 7 i (start at i=0/A, stop at
  i=6/B).  D lives in PSUM partitions 0:32, N in 32:64 -> the two matmul
  streams run on independent PE column-tiles.
- Tail per 512-chunk: reciprocal(D) -> DMA shift to N's partitions ->
  multiply -> DMA out.  bf16 everywhere between exp and the f32 division.
"""
import sys

sys.path.insert(0, "/opt/trn_rl_repo")

import numpy as np
import ml_dtypes

import concourse.bass as bass
import concourse.bacc as bacc
import concourse.tile as tile
from concourse import mybir
from concourse.bass_utils import run_bass_kernel_spmd

F32 = mybir.dt.float32
F32R = mybir.dt.float32r
BF16 = mybir.dt.bfloat16
AF = mybir.ActivationFunctionType
ALU = mybir.AluOpType

B, C, H, W = 4, 64, 64, 64
CO = 32
K = 7
PG = 70            # padded grid side
SP = H * W         # 4096 spatial
JA = 4             # bank A j-slots  (j = 0..3)
JB = 3             # bank B j-slots  (j = 4..6)


def build_graph():
    nc = bacc.Bacc(None, target_bir_lowering=False)
    xs_d = nc.dram_tensor("xs", [C, SP], F32, kind="ExternalInput")
    ys_d = nc.dram_tensor("ys", [C, SP], F32, kind="ExternalInput")
    wq_d = nc.dram_tensor("wq", [C, CO], F32, kind="ExternalInput")
    wk_d = nc.dram_tensor("wk", [C, CO], F32, kind="ExternalInput")
    wv_d = nc.dram_tensor("wv", [C, CO], F32, kind="ExternalInput")
    rel_d = nc.dram_tensor("rel", [CO, K], F32, kind="ExternalInput")
    bida_d = nc.dram_tensor("bida", [JA * CO, CO], BF16, kind="ExternalInput")
    bidb_d = nc.dram_tensor("bidb", [JB * CO, CO], BF16, kind="ExternalInput")
    out_d = nc.dram_tensor("out", [CO, SP], F32, kind="ExternalOutput")

    with tile.TileContext(nc) as tc:
        with (
            tc.tile_pool(name="sb", bufs=1) as sb,
            tc.tile_pool(name="tl", bufs=2) as tl,
            tc.tile_pool(name="ps", bufs=1, space="PSUM") as psacc,
        ):
            wq_sb = sb.tile([C, CO], F32R)
            wk_sb = sb.tile([C, CO], F32R)
            wv_sb = sb.tile([C, CO], F32R)
            bida = sb.tile([JA * CO, CO], BF16)
            bidb = sb.tile([JB * CO, CO], BF16)
            relA = sb.tile([JA * CO, K], F32)
            relB = sb.tile([JB * CO, K], F32)
            # j-pre-shifted padded grids; slot j holds grid shifted left by j
            kuA = sb.tile([JA * CO, PG, PG], BF16)
            kuB = sb.tile([JB * CO, PG, PG], BF16)
            vuA = sb.tile([JA * CO, PG, PG], BF16)
            vuB = sb.tile([JB * CO, PG, PG], BF16)
            qrA = sb.tile([JA * CO, SP], BF16)
            qrB = sb.tile([JB * CO, SP], BF16)
            PA = [sb.tile([JA * CO, SP], BF16, name=f"PA{t}", tag=f"PA{t}") for t in range(2)]
            PB = [sb.tile([JB * CO, SP], BF16, name=f"PB{t}", tag=f"PB{t}") for t in range(2)]
            EA = [sb.tile([JA * CO, SP], BF16, name=f"EA{t}", tag=f"EA{t}") for t in range(2)]
            EB = [sb.tile([JB * CO, SP], BF16, name=f"EB{t}", tag=f"EB{t}") for t in range(2)]
            EVA = [sb.tile([JA * CO, SP], BF16, name=f"EVA{t}", tag=f"EVA{t}") for t in range(2)]
            EVB = [sb.tile([JB * CO, SP], BF16, name=f"EVB{t}", tag=f"EVB{t}") for t in range(2)]
            acc = psacc.tile([128, SP], F32)  # D: 0:32, N: 32:64, scratch: 64:128

            # zero only the pad borders of the slot-0 grids; the j-shift
            # copies propagate them, and cols >= PG-j of slot j are never read
            for dst in (kuA, vuA):
                nc.gpsimd.memset(dst[0:CO, 0:3, :], 0.0)
                nc.gpsimd.memset(dst[0:CO, 3 + H:PG, :], 0.0)
                nc.gpsimd.memset(dst[0:CO, 3:3 + H, 0:3], 0.0)
                nc.gpsimd.memset(dst[0:CO, 3:3 + H, 3 + W:PG], 0.0)

            nc.sync.dma_start(out=wq_sb[:, :], in_=wq_d[:, :].bitcast(F32R))
            nc.sync.dma_start(out=wk_sb[:, :], in_=wk_d[:, :].bitcast(F32R))
            nc.sync.dma_start(out=wv_sb[:, :], in_=wv_d[:, :].bitcast(F32R))
            nc.sync.dma_start(out=bida[:, :], in_=bida_d[:, :])
            nc.sync.dma_start(out=bidb[:, :], in_=bidb_d[:, :])
            for j in range(JA):
                nc.sync.dma_start(out=relA[j * CO:(j + 1) * CO, :], in_=rel_d[:, :])
            for j in range(JB):
                nc.sync.dma_start(out=relB[j * CO:(j + 1) * CO, :], in_=rel_d[:, :])

            # ---------------- inputs + projections (PE f32), evac -> bf16
            # PSUM scratch: k/q use acc[64:96], v uses acc[96:128].
            with tc.tile_pool(name="io", bufs=1) as io:
                xs_sb = io.tile([C, SP], F32R)
                ys_sb = io.tile([C, SP], F32R)
                for cch in range(4):
                    sl = slice(cch * 1024, (cch + 1) * 1024)
                    nc.sync.dma_start(out=xs_sb[:, sl], in_=xs_d[:, sl].bitcast(F32R))
                    nc.sync.dma_start(out=ys_sb[:, sl], in_=ys_d[:, sl].bitcast(F32R))
                for (w_sb, src, kind, pbase) in (
                        (wq_sb, xs_sb, "q", 0), (wk_sb, xs_sb, "k", 0),
                        (wv_sb, ys_sb, "v", 0)):
                    pj = acc[pbase:pbase + CO, :]
                    for cch in range(8):
                        cs = slice(cch * 512, (cch + 1) * 512)
                        nc.tensor.matmul(pj[:, cs], w_sb[:, :], src[:, cs],
                                         start=True, stop=True,
                                         tile_position=(0, pbase))
                    # evac in 2 row-groups, each followed by its broadcast DMAs
                    for half_ in range(2):
                        r0, r1 = half_ * 32, half_ * 32 + 32
                        cs = slice(half_ * 2048, half_ * 2048 + 2048)
                        if kind == "q":
                            if half_ == 0:
                                nc.scalar.copy(out=qrA[0:CO, cs], in_=pj[:, cs])
                            else:
                                nc.vector.tensor_copy(qrA[0:CO, cs], pj[:, cs])
                            for j in range(1, K):
                                qd = qrA if j < JA else qrB
                                poff = (j * CO) if j < JA else ((j - JA) * CO)
                                nc.sync.dma_start(out=qd[poff:poff + CO, cs],
                                                  in_=qrA[0:CO, cs])
                        else:
                            dst = kuA if kind == "k" else vuA
                            dstB = kuB if kind == "k" else vuB
                            pr = pj[:, cs].rearrange("p (r w) -> p r w", w=W)
                            if half_ == 0:
                                nc.scalar.copy(out=dst[0:CO, 3 + r0:3 + r1, 3:3 + W], in_=pr)
                            else:
                                nc.vector.tensor_copy(dst[0:CO, 3 + r0:3 + r1, 3:3 + W], pr)
                            ra = r0 if half_ else 0          # include top pad rows once
                            rb = (3 + r1 + 3) if half_ else (3 + r1)
                            for j in range(1, K):
                                kd = dst if j < JA else dstB
                                poff = (j * CO) if j < JA else ((j - JA) * CO)
                                nc.sync.dma_start(
                                    out=kd[poff:poff + CO, ra:rb, 0:PG - j],
                                    in_=dst[0:CO, ra:rb, j:PG])

            # ---------------- main loop
            def grid(t, i, np_):
                # [np_, 64, 64] view of a padded grid at row shift i
                return t[0:np_, i:i + H, 0:W]

            def tight(t, np_):
                return t[0:np_, :].rearrange("p (r w) -> p r w", w=W)

            banks = (
                ("A", JA * CO, kuA, vuA, qrA, relA, PA, EA, EVA, bida),
                ("B", JB * CO, kuB, vuB, qrB, relB, PB, EB, EVB, bidb),
            )
            for i in range(K):
                for bi, (bn, np_, ku, vu, qr, rl, Pt, Et, EVt, bid) in enumerate(banks):
                    pb, eb, ev = Pt[i % 2], Et[i % 2], EVt[i % 2]
                    # fused logit: pb = (ku + rel_i) * q in one DVE op
                    nc.vector.scalar_tensor_tensor(
                        tight(pb, np_), ku[0:np_, i:i + H, 0:W],
                        rl[0:np_, i:i + 1], tight(qr, np_),
                        ALU.add, ALU.mult)
                    for hh in range(2):
                        hs = slice(hh * 2048, (hh + 1) * 2048)
                        nc.scalar.activation(eb[0:np_, hs], pb[0:np_, hs], AF.Exp)
                        nc.vector.tensor_tensor(
                            tight(ev, np_)[:, hh * 32:(hh + 1) * 32, :],
                            tight(eb, np_)[:, hh * 32:(hh + 1) * 32, :],
                            vu[0:np_, i + hh * 32:i + (hh + 1) * 32, 0:W], ALU.mult)
                    st = (i == 0 and bi == 0)
                    sp = (i == K - 1 and bi == len(banks) - 1)
                    for cch in range(8):
                        cs = slice(cch * 512, (cch + 1) * 512)
                        nc.tensor.matmul(acc[0:CO, cs], bid[:, :], eb[0:np_, cs],
                                         start=st, stop=sp, tile_position=(0, 0))
                        nc.tensor.matmul(acc[CO:2 * CO, cs], bid[:, :], ev[0:np_, cs],
                                         start=st, stop=sp, tile_position=(0, CO))

            # ---------------- per-chunk division tail
            for cch in range(8):
                cs = slice(cch * 512, (cch + 1) * 512)
                rcp_sb = tl.tile([CO, 512], F32, name="rcp_sb", tag="rcp_sb")
                rcp_mv = tl.tile([2 * CO, 512], F32, name="rcp_mv", tag="rcp_mv")
                outf = tl.tile([2 * CO, 512], F32, name="outf", tag="outf")
                nc.vector.reciprocal(rcp_sb[:, :], acc[0:CO, cs])
                nc.sync.dma_start(out=rcp_mv[CO:2 * CO, :], in_=rcp_sb[:, :])
                nc.vector.tensor_tensor(outf[CO:2 * CO, :], acc[CO:2 * CO, cs],
                                        rcp_mv[CO:2 * CO, :], ALU.mult)
                nc.sync.dma_start(out=out_d[:, cs], in_=outf[CO:2 * CO, :])
    nc.finalize()
    return nc


_nc_cache = None


def kernel(x, y, Wq, Wk, Wv, rel_h, rel_w, _trace=False):
    global _nc_cache
    if _nc_cache is None:
        _nc_cache = build_graph()
    nc = _nc_cache

    x = np.asarray(x, np.float32)
    y = np.asarray(y, np.float32)
    bf = ml_dtypes.bfloat16
    bida = np.ascontiguousarray(np.tile(np.eye(CO, dtype=np.float32), (JA, 1)).astype(bf))
    bidb = np.ascontiguousarray(np.tile(np.eye(CO, dtype=np.float32), (JB, 1)).astype(bf))
    in_maps = []
    for b in range(B):
        for half in range(2):
            sl = slice(half * CO, (half + 1) * CO)
            if half == 0:
                xs, ys = x[b], y[b]
                rel = np.asarray(rel_h, np.float32)
            else:
                xs = np.ascontiguousarray(x[b].transpose(0, 2, 1))
                ys = np.ascontiguousarray(y[b].transpose(0, 2, 1))
                rel = np.asarray(rel_w, np.float32)
            in_maps.append({
                "xs": np.ascontiguousarray(xs.reshape(C, SP)),
                "ys": np.ascontiguousarray(ys.reshape(C, SP)),
                "wq": np.ascontiguousarray(np.asarray(Wq, np.float32)[sl].T),
                "wk": np.ascontiguousarray(np.asarray(Wk, np.float32)[sl].T),
                "wv": np.ascontiguousarray(np.asarray(Wv, np.float32)[sl].T),
                "rel": np.ascontiguousarray(rel),
                "bida": bida,
                "bidb": bidb,
            })

    res = run_bass_kernel_spmd(nc, in_maps, core_ids=list(range(8)), trace=_trace)

    out = np.empty((B, 2 * CO, H, W), np.float32)
    idx = 0
    for b in range(B):
        for half in range(2):
            o = res.results[idx]["out"].reshape(CO, H, W)
            if half == 1:
                o = o.transpose(0, 2, 1)
            out[b, half * CO:(half + 1) * CO] = o
            idx += 1
    if _trace:
        return out, res
    return out



# revision 14
# speedup vs baseline: 1.1122x; 1.1122x over previous
"""v3: PE-accumulated window sums ("layout B", j-banks variant).

Same sharding and math as v2 (see kernel.py), but the window reduction
Sum_t E_t and Sum_t E_t*v_t runs on the TensorEngine instead of DVE adds:

- Partitions = j-slot x 32 channels.  Bank A: j in 0..3 (128 partitions),
  bank B: j in 4..6 (96 partitions).
- k_unf/v_unf hold the j-pre-shifted padded [70x70] grids, so window offset
  (i, j) is a pure row shift (i) of partition (j, c)'s grid.
- Per i and bank: one fused STT logit op, one batched exp (ACT), one E*v
  multiply, then 16 block-identity matmuls (8 chunks x {D, N}) that
  accumulate straight into PSUM across all 7 i (start at i=0/A, stop at
  i=6/B).  D lives in PSUM partitions 0:32, N in 32:64 -> the two matmul
  streams run on independent PE column-tiles.
- Tail per 512-chunk: reciprocal(D) -> DMA shift to N's partitions ->
  multiply -> DMA out.  bf16 everywhere between exp and the f32 division.
"""
import sys

sys.path.insert(0, "/opt/trn_rl_repo")

import numpy as np
import ml_dtypes

import concourse.bass as bass
import concourse.bacc as bacc
import concourse.tile as tile
from concourse import mybir
from concourse.bass_utils import run_bass_kernel_spmd

F32 = mybir.dt.float32
F32R = mybir.dt.float32r
BF16 = mybir.dt.bfloat16
AF = mybir.ActivationFunctionType
ALU = mybir.AluOpType

B, C, H, W = 4, 64, 64, 64
CO = 32
K = 7
PG = 70            # padded grid side
SP = H * W         # 4096 spatial
JA = 4             # bank A j-slots  (j = 0..3)
JB = 3             # bank B j-slots  (j = 4..6)


def build_graph():
    nc = bacc.Bacc(None, target_bir_lowering=False)
    xs_d = nc.dram_tensor("xs", [C, SP], F32, kind="ExternalInput")
    ys_d = nc.dram_tensor("ys", [C, SP], F32, kind="ExternalInput")
    wq_d = nc.dram_tensor("wq", [C, CO], F32, kind="ExternalInput")
    wk_d = nc.dram_tensor("wk", [C, CO], F32, kind="ExternalInput")
    wv_d = nc.dram_tensor("wv", [C, CO], F32, kind="ExternalInput")
    rel_d = nc.dram_tensor("rel", [CO, K], F32, kind="ExternalInput")
    bida_d = nc.dram_tensor("bida", [JA * CO, CO], BF16, kind="ExternalInput")
    bidb_d = nc.dram_tensor("bidb", [JB * CO, CO], BF16, kind="ExternalInput")
    out_d = nc.dram_tensor("out", [CO, SP], F32, kind="ExternalOutput")

    with tile.TileContext(nc) as tc:
        with (
            tc.tile_pool(name="sb", bufs=1) as sb,
            tc.tile_pool(name="tl", bufs=2) as tl,
            tc.tile_pool(name="ps", bufs=1, space="PSUM") as psacc,
        ):
            wq_sb = sb.tile([C, CO], F32R)
            wk_sb = sb.tile([C, CO], F32R)
            wv_sb = sb.tile([C, CO], F32R)
            bida = sb.tile([JA * CO, CO], BF16)
            bidb = sb.tile([JB * CO, CO], BF16)
            relA = sb.tile([JA * CO, K], F32)
            relB = sb.tile([JB * CO, K], F32)
            # j-pre-shifted padded grids; slot j holds grid shifted left by j
            kuA = sb.tile([JA * CO, PG, PG], BF16)
            kuB = sb.tile([JB * CO, PG, PG], BF16)
            vuA = sb.tile([JA * CO, PG, PG], BF16)
            vuB = sb.tile([JB * CO, PG, PG], BF16)
            qrA = sb.tile([JA * CO, SP], BF16)
            qrB = sb.tile([JB * CO, SP], BF16)
            PA = [sb.tile([JA * CO, SP], BF16, name=f"PA{t}", tag=f"PA{t}") for t in range(2)]
            PB = [sb.tile([JB * CO, SP], BF16, name=f"PB{t}", tag=f"PB{t}") for t in range(2)]
            EA = [sb.tile([JA * CO, SP], BF16, name=f"EA{t}", tag=f"EA{t}") for t in range(2)]
            EB = [sb.tile([JB * CO, SP], BF16, name=f"EB{t}", tag=f"EB{t}") for t in range(2)]
            EVA = [sb.tile([JA * CO, SP], BF16, name=f"EVA{t}", tag=f"EVA{t}") for t in range(2)]
            EVB = [sb.tile([JB * CO, SP], BF16, name=f"EVB{t}", tag=f"EVB{t}") for t in range(2)]
            kbA = sb.tile([JA * CO, SP], BF16)
            acc = psacc.tile([128, SP], F32)  # D: 0:32, N: 32:64, scratch: 64:128

            # zero only the pad borders of the slot-0 grids; the j-shift
            # copies propagate them, and cols >= PG-j of slot j are never read
            for dst in (kuA, vuA):
                nc.gpsimd.memset(dst[0:CO, 0:3, :], 0.0)
                nc.gpsimd.memset(dst[0:CO, 3 + H:PG, :], 0.0)
                nc.gpsimd.memset(dst[0:CO, 3:3 + H, 0:3], 0.0)
                nc.gpsimd.memset(dst[0:CO, 3:3 + H, 3 + W:PG], 0.0)

            nc.sync.dma_start(out=wq_sb[:, :], in_=wq_d[:, :].bitcast(F32R))
            nc.sync.dma_start(out=wk_sb[:, :], in_=wk_d[:, :].bitcast(F32R))
            nc.sync.dma_start(out=wv_sb[:, :], in_=wv_d[:, :].bitcast(F32R))
            nc.sync.dma_start(out=bida[:, :], in_=bida_d[:, :])
            nc.sync.dma_start(out=bidb[:, :], in_=bidb_d[:, :])
            for j in range(JA):
                nc.sync.dma_start(out=relA[j * CO:(j + 1) * CO, :], in_=rel_d[:, :])
            for j in range(JB):
                nc.sync.dma_start(out=relB[j * CO:(j + 1) * CO, :], in_=rel_d[:, :])

            # ---------------- inputs + projections (PE f32), evac -> bf16
            # PSUM scratch: k/q use acc[64:96], v uses acc[96:128].
            with tc.tile_pool(name="io", bufs=1) as io:
                xs_sb = io.tile([C, SP], F32R)
                ys_sb = io.tile([C, SP], F32R)
                for cch in range(4):
                    sl = slice(cch * 1024, (cch + 1) * 1024)
                    nc.sync.dma_start(out=xs_sb[:, sl], in_=xs_d[:, sl].bitcast(F32R))
                    nc.sync.dma_start(out=ys_sb[:, sl], in_=ys_d[:, sl].bitcast(F32R))
                for (w_sb, src, kind, pbase) in (
                        (wq_sb, xs_sb, "q", 0), (wk_sb, xs_sb, "k", 0),
                        (wv_sb, ys_sb, "v", 0)):
                    pj = acc[pbase:pbase + CO, :]
                    for cch in range(8):
                        cs = slice(cch * 512, (cch + 1) * 512)
                        nc.tensor.matmul(pj[:, cs], w_sb[:, :], src[:, cs],
                                         start=True, stop=True,
                                         tile_position=(0, pbase))
                    # evac in 2 row-groups, each followed by its broadcast DMAs
                    for half_ in range(2):
                        r0, r1 = half_ * 32, half_ * 32 + 32
                        cs = slice(half_ * 2048, half_ * 2048 + 2048)
                        if kind == "q":
                            if half_ == 0:
                                nc.scalar.copy(out=qrA[0:CO, cs], in_=pj[:, cs])
                            else:
                                nc.vector.tensor_copy(qrA[0:CO, cs], pj[:, cs])
                            for j in range(1, K):
                                qd = qrA if j < JA else qrB
                                poff = (j * CO) if j < JA else ((j - JA) * CO)
                                nc.sync.dma_start(out=qd[poff:poff + CO, cs],
                                                  in_=qrA[0:CO, cs])
                        else:
                            dst = kuA if kind == "k" else vuA
                            dstB = kuB if kind == "k" else vuB
                            pr = pj[:, cs].rearrange("p (r w) -> p r w", w=W)
                            if half_ == 0:
                                nc.scalar.copy(out=dst[0:CO, 3 + r0:3 + r1, 3:3 + W], in_=pr)
                            else:
                                nc.vector.tensor_copy(dst[0:CO, 3 + r0:3 + r1, 3:3 + W], pr)
                            ra = r0 if half_ else 0          # include top pad rows once
                            rb = (3 + r1 + 3) if half_ else (3 + r1)
                            for j in range(1, K):
                                kd = dst if j < JA else dstB
                                poff = (j * CO) if j < JA else ((j - JA) * CO)
                                nc.sync.dma_start(
                                    out=kd[poff:poff + CO, ra:rb, 0:PG - j],
                                    in_=dst[0:CO, ra:rb, j:PG])

            # ---------------- main loop
            def grid(t, i, np_):
                # [np_, 64, 64] view of a padded grid at row shift i
                return t[0:np_, i:i + H, 0:W]

            def tight(t, np_):
                return t[0:np_, :].rearrange("p (r w) -> p r w", w=W)

            banks = (
                ("A", JA * CO, kuA, vuA, qrA, relA, PA, EA, EVA, bida),
                ("B", JB * CO, kuB, vuB, qrB, relB, PB, EB, EVB, bidb),
            )
            for i in range(K):
                for bi, (bn, np_, ku, vu, qr, rl, Pt, Et, EVt, bid) in enumerate(banks):
                    pb, eb, ev = Pt[i % 2], Et[i % 2], EVt[i % 2]
                    kb = kbA  # DVE-only scratch; producer+consumer in-order on DVE
                    nc.vector.tensor_scalar_add(
                        tight(kb, np_), ku[0:np_, i:i + H, 0:W], rl[0:np_, i:i + 1])
                    nc.vector.tensor_tensor(
                        tight(pb, np_), tight(kb, np_), tight(qr, np_), ALU.mult)
                    for hh in range(2):
                        hs = slice(hh * 2048, (hh + 1) * 2048)
                        nc.scalar.activation(eb[0:np_, hs], pb[0:np_, hs], AF.Exp)
                        # E*v: DVE takes 3 of 4 KB, Pool takes 1 KB
                        nc.vector.tensor_tensor(
                            tight(ev, np_)[:, hh * 32:hh * 32 + 24, :],
                            tight(eb, np_)[:, hh * 32:hh * 32 + 24, :],
                            vu[0:np_, i + hh * 32:i + hh * 32 + 24, 0:W], ALU.mult)
                        nc.gpsimd.tensor_tensor(
                            tight(ev, np_)[:, hh * 32 + 24:hh * 32 + 32, :],
                            tight(eb, np_)[:, hh * 32 + 24:hh * 32 + 32, :],
                            vu[0:np_, i + hh * 32 + 24:i + hh * 32 + 32, 0:W], ALU.mult)
                    st = (i == 0 and bi == 0)
                    sp = (i == K - 1 and bi == len(banks) - 1)
                    for cch in range(8):
                        cs = slice(cch * 512, (cch + 1) * 512)
                        nc.tensor.matmul(acc[0:CO, cs], bid[:, :], eb[0:np_, cs],
                                         start=st, stop=sp, tile_position=(0, 0))
                        nc.tensor.matmul(acc[CO:2 * CO, cs], bid[:, :], ev[0:np_, cs],
                                         start=st, stop=sp, tile_position=(0, CO))

            # ---------------- per-chunk division tail
            for cch in range(8):
                cs = slice(cch * 512, (cch + 1) * 512)
                rcp_sb = tl.tile([CO, 512], F32, name="rcp_sb", tag="rcp_sb")
                rcp_mv = tl.tile([2 * CO, 512], F32, name="rcp_mv", tag="rcp_mv")
                outf = tl.tile([2 * CO, 512], F32, name="outf", tag="outf")
                nc.vector.reciprocal(rcp_sb[:, :], acc[0:CO, cs])
                nc.sync.dma_start(out=rcp_mv[CO:2 * CO, :], in_=rcp_sb[:, :])
                nc.vector.tensor_tensor(outf[CO:2 * CO, :], acc[CO:2 * CO, cs],
                                        rcp_mv[CO:2 * CO, :], ALU.mult)
                nc.sync.dma_start(out=out_d[:, cs], in_=outf[CO:2 * CO, :])
    nc.finalize()
    return nc


_nc_cache = None


def kernel(x, y, Wq, Wk, Wv, rel_h, rel_w, _trace=False):
    global _nc_cache
    if _nc_cache is None:
        _nc_cache = build_graph()
    nc = _nc_cache

    x = np.asarray(x, np.float32)
    y = np.asarray(y, np.float32)
    bf = ml_dtypes.bfloat16
    bida = np.ascontiguousarray(np.tile(np.eye(CO, dtype=np.float32), (JA, 1)).astype(bf))
    bidb = np.ascontiguousarray(np.tile(np.eye(CO, dtype=np.float32), (JB, 1)).astype(bf))
    in_maps = []
    for b in range(B):
        for half in range(2):
            sl = slice(half * CO, (half + 1) * CO)
            if half == 0:
                xs, ys = x[b], y[b]
                rel = np.asarray(rel_h, np.float32)
            else:
                xs = np.ascontiguousarray(x[b].transpose(0, 2, 1))
                ys = np.ascontiguousarray(y[b].transpose(0, 2, 1))
                rel = np.asarray(rel_w, np.float32)
            in_maps.append({
                "xs": np.ascontiguousarray(xs.reshape(C, SP)),
                "ys": np.ascontiguousarray(ys.reshape(C, SP)),
                "wq": np.ascontiguousarray(np.asarray(Wq, np.float32)[sl].T),
                "wk": np.ascontiguousarray(np.asarray(Wk, np.float32)[sl].T),
                "wv": np.ascontiguousarray(np.asarray(Wv, np.float32)[sl].T),
                "rel": np.ascontiguousarray(rel),
                "bida": bida,
                "bidb": bidb,
            })

    res = run_bass_kernel_spmd(nc, in_maps, core_ids=list(range(8)), trace=_trace)

    out = np.empty((B, 2 * CO, H, W), np.float32)
    idx = 0
    for b in range(B):
        for half in range(2):
            o = res.results[idx]["out"].reshape(CO, H, W)
            if half == 1:
                o = o.transpose(0, 2, 1)
            out[b, half * CO:(half + 1) * CO] = o
            idx += 1
    if _trace:
        return out, res
    return out



# revision 18
# speedup vs baseline: 1.4596x; 1.3124x over previous
"""v4: spatial-quarters layout.

Partitions = 4 spatial quarters x 32 channels (always 128, fully packed).
k/v live as per-quarter haloed padded grids kg/vg [128, 22, 70]; tap (i, j)
is the pure strided slice [:, i:i+16, j:j+64].  q is quartered to [128, 1024].

Per (i, j-group) iteration (j-groups: j=0..3 and j=4..6):
- one batched DVE tensor_scalar add (kb = kg_taps + rel_i) over an
  overlapping custom AP covering all nj j-offsets at once,
- one batched DVE multiply pb = kb * q (q broadcast over j via stride-0 AP),
  (or one fused Pool scalar_tensor_tensor for offloaded groups),
- one ACT exp over the whole group,
- one batched DVE/Pool multiply ev = eb * vg_taps,
- per tap: 4 identity matmuls (2x512 cols for each of SE/SEV) accumulating
  straight into PSUM over all 49 taps.

Tail: out = SEV * reciprocal(SE) on the same partitions (no DMA hop), 4
quarter DMAs out.  Projections run on PE in float32r (1 cycle/row).

Sharding: 8 cores = batch(4) x channel-half(2); the half=1 core sees
H/W-transposed inputs so its rel_w bias becomes a row (i) bias too.
"""
import sys

sys.path.insert(0, "/opt/trn_rl_repo")

import numpy as np
import ml_dtypes

import bass_rust
import concourse.bass as bass
import concourse.bacc as bacc
import concourse.tile as tile
from concourse import mybir
from concourse.bass_utils import run_bass_kernel_spmd

F32 = mybir.dt.float32
F32R = mybir.dt.float32r
BF16 = mybir.dt.bfloat16
AF = mybir.ActivationFunctionType
ALU = mybir.AluOpType

B, C, H, W = 4, 64, 64, 64
CO = 32
K = 7
PG = 70            # padded grid side
SP = H * W         # 4096
NQ = 4             # spatial quarters
QR = H // NQ       # 16 rows per quarter
QS = QR * W        # 1024 spatial per quarter
GR = QR + K - 1    # 22 haloed grid rows per quarter

# (i, g) groups whose logit / E*v work runs on Pool instead of DVE
# (Pool has no TensorScalarPtr/STT support on TRN2 ISA — only plain TT)
POOL_LOGIT = set()
POOL_EV = {(i, 1) for i in range(K)}


def _sp_view(t, sl):
    """[128, 16, 64] view of a quarter-spatial slice of a flat tile."""
    return t[:, sl].rearrange("p (r w) -> p r w", w=W)


def build_graph():
    nc = bacc.Bacc(None, target_bir_lowering=False)
    xs_d = nc.dram_tensor("xs", [C, SP], F32, kind="ExternalInput")
    ys_d = nc.dram_tensor("ys", [C, SP], F32, kind="ExternalInput")
    wq_d = nc.dram_tensor("wq", [C, CO], F32, kind="ExternalInput")
    wk_d = nc.dram_tensor("wk", [C, CO], F32, kind="ExternalInput")
    wv_d = nc.dram_tensor("wv", [C, CO], F32, kind="ExternalInput")
    rel_d = nc.dram_tensor("rel", [CO, K], F32, kind="ExternalInput")
    id_d = nc.dram_tensor("id128", [128, 128], BF16, kind="ExternalInput")
    out_d = nc.dram_tensor("out", [CO, SP], F32, kind="ExternalOutput")

    with tile.TileContext(nc) as tc:
        with (
            tc.tile_pool(name="sb", bufs=1) as sb,
            tc.tile_pool(name="ps", bufs=1, space="PSUM") as psp,
        ):
            wq_sb = sb.tile([C, CO], F32R)
            wk_sb = sb.tile([C, CO], F32R)
            wv_sb = sb.tile([C, CO], F32R)
            rlq = sb.tile([128, K], F32)
            idm = sb.tile([128, 128], BF16)
            kg = sb.tile([128, GR, PG], BF16)
            vg = sb.tile([128, GR, PG], BF16)
            qq = sb.tile([128, QS], BF16)
            kstg = sb.tile([CO, PG, PG], BF16)
            vstg = sb.tile([CO, PG, PG], BF16)
            qstg = sb.tile([CO, SP], BF16)
            kb = sb.tile([128, NQ * QS], BF16)
            PB = [sb.tile([128, NQ * QS], BF16, name=f"PB{t}", tag=f"PB{t}") for t in range(2)]
            EB = [sb.tile([128, NQ * QS], BF16, name=f"EB{t}", tag=f"EB{t}") for t in range(2)]
            EVt = [sb.tile([128, NQ * QS], BF16, name=f"EV{t}", tag=f"EV{t}") for t in range(2)]
            rcp = sb.tile([128, QS], F32)
            outf = sb.tile([128, QS], F32)
            SE = psp.tile([128, QS], F32)
            SEV = psp.tile([128, QS], F32)
            PRJ = [psp.tile([128, 512], F32, name=f"PRJ{t}", tag=f"PRJ{t}") for t in range(2)]

            nc.sync.dma_start(out=wk_sb[:, :], in_=wk_d[:, :].bitcast(F32R))
            nc.sync.dma_start(out=wq_sb[:, :], in_=wq_d[:, :].bitcast(F32R))
            nc.sync.dma_start(out=wv_sb[:, :], in_=wv_d[:, :].bitcast(F32R))
            nc.sync.dma_start(out=idm[:, :], in_=id_d[:, :])
            for s in range(NQ):
                nc.sync.dma_start(out=rlq[32 * s:32 * s + 32, :], in_=rel_d[:, :])

            xs_sb = sb.tile([C, SP], F32R)
            ys_sb = sb.tile([C, SP], F32R)
            for cch in range(4):
                sl = slice(cch * 1024, (cch + 1) * 1024)
                nc.sync.dma_start(out=xs_sb[:, sl], in_=xs_d[:, sl].bitcast(F32R))
            for cch in range(4):
                sl = slice(cch * 1024, (cch + 1) * 1024)
                nc.sync.dma_start(out=ys_sb[:, sl], in_=ys_d[:, sl].bitcast(F32R))

            # pad borders of the staging grids
            for stg in (kstg, vstg):
                nc.gpsimd.memset(stg[:, 0:3, :], 0.0)
                nc.gpsimd.memset(stg[:, 3 + H:PG, :], 0.0)
                nc.gpsimd.memset(stg[:, 3:3 + H, 0:3], 0.0)
                nc.gpsimd.memset(stg[:, 3:3 + H, 3 + W:PG], 0.0)

            # projections (f32r, dst partition 0) + evac + quarter broadcasts
            for (w_sb, src, kind) in ((wk_sb, xs_sb, "k"), (wq_sb, xs_sb, "q"),
                                      (wv_sb, ys_sb, "v")):
                for cch in range(8):
                    cs = slice(cch * 512, (cch + 1) * 512)
                    pj = PRJ[cch % 2]
                    nc.tensor.matmul(pj[0:CO, :], w_sb[:, :], src[:, cs],
                                     start=True, stop=True)
                    eng = nc.scalar.copy if cch % 2 == 0 else (
                        lambda out, in_: nc.vector.tensor_copy(out, in_))
                    if kind == "q":
                        eng(out=qstg[:, cs], in_=pj[0:CO, :])
                    else:
                        stg = kstg if kind == "k" else vstg
                        eng(out=stg[:, 3 + 8 * cch:3 + 8 * cch + 8, 3:3 + W],
                            in_=pj[0:CO, :].rearrange("p (r w) -> p r w", w=W))
                if kind == "q":
                    for s in range(NQ):
                        nc.sync.dma_start(
                            out=qq[32 * s:32 * s + 32, :],
                            in_=qstg[:, s * QS:(s + 1) * QS])
                else:
                    dst = kg if kind == "k" else vg
                    stg = kstg if kind == "k" else vstg
                    for s in range(NQ):
                        nc.sync.dma_start(
                            out=dst[32 * s:32 * s + 32, :, :],
                            in_=stg[:, QR * s:QR * s + GR, :])

            # ---------------- main loop over row-taps i and j-groups
            for i in range(K):
                for g, (j0, nj) in enumerate(((0, 4), (4, 3))):
                    t = (2 * i + g) % 2
                    pb, eb, ev = PB[t], EB[t], EVt[t]
                    qqv = _sp_view(qq, slice(0, QS))
                    for jj in range(nj):
                        j = j0 + jj
                        sl = slice(jj * QS, (jj + 1) * QS)
                        if (i, g) in POOL_LOGIT:
                            nc.gpsimd.scalar_tensor_tensor(
                                _sp_view(pb, sl), kg[:, i:i + QR, j:j + W],
                                rlq[:, i:i + 1], qqv, ALU.add, ALU.mult)
                        else:
                            nc.vector.tensor_scalar_add(
                                _sp_view(kb, sl), kg[:, i:i + QR, j:j + W],
                                rlq[:, i:i + 1])
                            nc.vector.tensor_tensor(
                                _sp_view(pb, sl), _sp_view(kb, sl), qqv, ALU.mult)
                    nc.scalar.activation(eb[:, 0:nj * QS], pb[:, 0:nj * QS], AF.Exp)
                    for jj in range(nj):
                        j = j0 + jj
                        sl = slice(jj * QS, (jj + 1) * QS)
                        eng = nc.gpsimd if (i, g) in POOL_EV else nc.vector
                        eng.tensor_tensor(
                            _sp_view(ev, sl), _sp_view(eb, sl),
                            vg[:, i:i + QR, j:j + W], ALU.mult)
                    for jj in range(nj):
                        st = (i == 0 and j0 + jj == 0)
                        sp = (i == K - 1 and j0 + jj == K - 1)
                        for hh in range(2):
                            cs = slice(jj * QS + hh * 512, jj * QS + hh * 512 + 512)
                            hs = slice(hh * 512, hh * 512 + 512)
                            nc.tensor.matmul(SE[:, hs], idm[:, :], eb[:, cs],
                                             start=st, stop=sp)
                            nc.tensor.matmul(SEV[:, hs], idm[:, :], ev[:, cs],
                                             start=st, stop=sp)

            # ---------------- division tail + output
            for hh in range(2):
                hs = slice(hh * 512, hh * 512 + 512)
                nc.vector.reciprocal(rcp[:, hs], SE[:, hs])
                nc.vector.tensor_tensor(outf[:, hs], SEV[:, hs], rcp[:, hs],
                                        ALU.mult)
            for s in range(NQ):
                nc.sync.dma_start(out=out_d[:, s * QS:(s + 1) * QS],
                                  in_=outf[32 * s:32 * s + 32, :])
    nc.finalize()
    return nc


_nc_cache = None


def kernel(x, y, Wq, Wk, Wv, rel_h, rel_w, _trace=False):
    global _nc_cache
    if _nc_cache is None:
        _nc_cache = build_graph()
    nc = _nc_cache

    x = np.asarray(x, np.float32)
    y = np.asarray(y, np.float32)
    bf = ml_dtypes.bfloat16
    id128 = np.ascontiguousarray(np.eye(128, dtype=np.float32).astype(bf))
    in_maps = []
    for b in range(B):
        for half in range(2):
            sl = slice(half * CO, (half + 1) * CO)
            if half == 0:
                xs, ys = x[b], y[b]
                rel = np.asarray(rel_h, np.float32)
            else:
                xs = np.ascontiguousarray(x[b].transpose(0, 2, 1))
                ys = np.ascontiguousarray(y[b].transpose(0, 2, 1))
                rel = np.asarray(rel_w, np.float32)
            in_maps.append({
                "xs": np.ascontiguousarray(xs.reshape(C, SP)),
                "ys": np.ascontiguousarray(ys.reshape(C, SP)),
                "wq": np.ascontiguousarray(np.asarray(Wq, np.float32)[sl].T),
                "wk": np.ascontiguousarray(np.asarray(Wk, np.float32)[sl].T),
                "wv": np.ascontiguousarray(np.asarray(Wv, np.float32)[sl].T),
                "rel": np.ascontiguousarray(rel),
                "id128": id128,
            })

    res = run_bass_kernel_spmd(nc, in_maps, core_ids=list(range(8)), trace=_trace)

    out = np.empty((B, 2 * CO, H, W), np.float32)
    idx = 0
    for b in range(B):
        for half in range(2):
            o = res.results[idx]["out"].reshape(CO, H, W)
            if half == 1:
                o = o.transpose(0, 2, 1)
            out[b, half * CO:(half + 1) * CO] = o
            idx += 1
    if _trace:
        return out, res
    return out
